# revision 10
# baseline (speedup 1.0000x reference)
"""Distributed RGCN+GraphConv (gated residual) kernel for 8 Trainium2 cores.

Sharding: target nodes are bin-packed into bins of <=16 nodes whose total
in-degree is <=256.  Each core owns BINS_C consecutive bins (graph/data
parallel over targets).  Edge lists are padded per-bin to a uniform structure
so a single SPMD NEFF serves all cores.

v3: Layer-2 source features are gathered from an SBUF-resident copy of the
AllGathered h1 table (SBUF-source transposed dma_gather) instead of per-edge
random reads from DRAM, which were HBM-latency-bound (~8.7us per 1024 rows).
The feature-major gathered tiles are flipped to edge-major with PE
transposes before the scatter-matmul aggregation.  The h1 table is stored
blocked ([group][core][partition][row][feat]) so the own-store, AllGather
and table->SBUF loads all use wide descriptors.  L1 keeps the v2 structure
(host-pregathered xs1 stream + one-hot scatter matmuls); x/h1 residuals now
run in bf16 to free SBUF for the table.
"""

import numpy as np
import ml_dtypes

import concourse.bacc as bacc
import concourse.mybir as mybir
import concourse.tile as tile
from concourse.library_config import mlp as _mlp_lib
from concourse.bass_utils import run_bass_kernel_spmd

BF16 = ml_dtypes.bfloat16

N = 20000
E = 320000
R = 8
G = 256          # feature width (g_dim == h1_dim == h2_dim)
CORES = 8
P = 128
SLOT = 16        # target slots per bin
CAP = 256        # edge slots per bin (2 chunks of 128)
TG = 512         # targets per tile-group
BINS_TG = TG // SLOT          # 32 bins per tile-group

F32 = mybir.dt.float32
BF = mybir.dt.bfloat16
I16 = mybir.dt.int16

_nc_cache: dict = {}
L2_SRC = "dram"   # "sbuf": gather h1 rows from SBUF table; "dram": from DRAM
NTG_PRE = 3       # AG groups readable by each block's first gather call
GSZ = 512         # idxs per SBUF-source gather call (transpose rx-desc limit)


# ----------------------------------------------------------------------------
# host-side: bin packing of target nodes
# ----------------------------------------------------------------------------

def _pack_bins(deg: np.ndarray, bins_c: int):
    """LPT pack nodes into CORES*bins_c bins (<=SLOT nodes, <=CAP edge sum).

    Returns (bin_of_node, slot_in_bin) or None if infeasible."""
    import heapq

    nbins = CORES * bins_c
    order = np.argsort(-deg, kind="stable")
    heap = [(0, b) for b in range(nbins)]
    heapq.heapify(heap)
    counts = np.zeros(nbins, np.int32)
    sums = np.zeros(nbins, np.int64)
    bin_of = np.full(N, -1, np.int32)
    slot_of = np.full(N, -1, np.int32)
    stash = []
    for n in order:
        d = int(deg[n])
        placed = False
        while heap:
            s, b = heapq.heappop(heap)
            if counts[b] >= SLOT:
                continue        # bin full by count; drop from heap
            if s + d > CAP:
                stash.append((s, b))
                # smallest-sum bin can't take it -> no bin can (heap is by sum)
                break
            bin_of[n] = b
            slot_of[n] = counts[b]
            counts[b] += 1
            sums[b] = s + d
            if counts[b] < SLOT:
                heapq.heappush(heap, (int(sums[b]), b))
            placed = True
            break
        for item in stash:
            heapq.heappush(heap, item)
        stash.clear()
        if not placed:
            return None
    return bin_of, slot_of


# ----------------------------------------------------------------------------
# device kernel builder (structure depends only on bins_c)
# ----------------------------------------------------------------------------

def _build_nc(bins_c: int):
    t_c = bins_c * SLOT              # targets per core
    npad = bins_c * CAP              # edge slots per core
    ncol = npad // P                 # chunk columns
    ntg = t_c // TG                  # tile groups
    nidxcol = npad // 16
    tabcols = CORES * t_c // P       # SBUF table ranks (512B each)

    nc = bacc.Bacc("TRN2", target_bir_lowering=False, debug=False,
                   num_devices=CORES)

    t_xs1 = nc.dram_tensor("xs1", [ncol // 8, P, 8 * G], BF,
                           kind="ExternalInput")
    t_s1 = nc.dram_tensor("s1m", [ncol // 8, P, 8 * P], BF,
                          kind="ExternalInput")
    t_s2 = nc.dram_tensor("s2m", [ncol // 8, P, 8 * P], BF,
                          kind="ExternalInput")
    t_xT = nc.dram_tensor("xT", [P, 2, t_c], BF, kind="ExternalInput")
    t_idx2 = nc.dram_tensor("idx2", [P, nidxcol], I16, kind="ExternalInput")
    t_wfull = nc.dram_tensor("wfull", [P, 16, G], BF, kind="ExternalInput")
    t_root1 = nc.dram_tensor("root1", [P, 2, G], BF, kind="ExternalInput")
    t_g1w = nc.dram_tensor("g1w", [P, 4, G], BF, kind="ExternalInput")
    t_wrel = nc.dram_tensor("wrel", [P, 2, G], BF, kind="ExternalInput")
    t_wroot = nc.dram_tensor("wroot", [P, 2, G], BF, kind="ExternalInput")
    t_g2w = nc.dram_tensor("g2w", [P, 4, G], BF, kind="ExternalInput")
    t_bias = nc.dram_tensor("biases", [P, 8], F32, kind="ExternalInput")
    t_ident = nc.dram_tensor("ident", [P, P], BF, kind="ExternalInput")

    t_out = nc.dram_tensor("h2T", [2, P, t_c], F32, kind="ExternalOutput")

    # blocked h1 table: own [group][128][4 rows][G], gathered
    # [group][core][128][4 rows][G] so every DRAM touch is >=2KB/partition
    d_ownB = nc.dram_tensor("h1_ownB", [ntg, P, TG // P, G], BF)
    d_tabB = nc.dram_tensor("h1_tabB", [ntg, CORES, P, TG // P, G], BF,
                            addr_space="Shared")

    Iden = mybir.ActivationFunctionType.Identity
    Sigm = mybir.ActivationFunctionType.Sigmoid
    MUL = mybir.AluOpType.mult
    SUB = mybir.AluOpType.subtract
    ADD = mybir.AluOpType.add

    with tile.TileContext(nc, num_cores=CORES) as tc:
        with tc.tile_pool(name="cst", bufs=1) as cst, \
             tc.tile_pool(name="res", bufs=1) as res, \
             tc.tile_pool(name="pA", bufs=2) as pA, \
             tc.tile_pool(name="pG", bufs=3) as pG, \
             tc.tile_pool(name="pG2", bufs=12) as pG2, \
             tc.tile_pool(name="pEM", bufs=4) as pEM, \
             tc.tile_pool(name="pS", bufs=5) as pS, \
             tc.tile_pool(name="pH", bufs=2) as pH, \
             tc.tile_pool(name="psA", bufs=2, space="PSUM") as psA, \
             tc.tile_pool(name="psD", bufs=2, space="PSUM") as psD, \
             tc.tile_pool(name="psT", bufs=2, space="PSUM") as psT:

            nc.gpsimd.load_library(_mlp_lib)

            # ------- load constants / weights (Act-engine HWDGE ring so
            # they do not head-of-line block the Sync ring's edge streams) ---
            idx2_t = cst.tile([P, nidxcol], I16)
            nc.scalar.dma_start(out=idx2_t[:], in_=t_idx2[:])
            wfull_t = cst.tile([P, 16, G], BF)
            nc.scalar.dma_start(out=wfull_t[:], in_=t_wfull[:])
            root1_t = cst.tile([P, 2, G], BF)
            nc.scalar.dma_start(out=root1_t[:], in_=t_root1[:])
            g1w_t = cst.tile([P, 4, G], BF)
            nc.scalar.dma_start(out=g1w_t[:], in_=t_g1w[:])
            wrel_t = cst.tile([P, 2, G], BF)
            nc.scalar.dma_start(out=wrel_t[:], in_=t_wrel[:])
            wroot_t = cst.tile([P, 2, G], BF)
            nc.scalar.dma_start(out=wroot_t[:], in_=t_wroot[:])
            g2w_t = cst.tile([P, 4, G], BF)
            nc.scalar.dma_start(out=g2w_t[:], in_=t_g2w[:])
            bias_t = cst.tile([P, 8], F32)
            nc.scalar.dma_start(out=bias_t[:], in_=t_bias[:])
            ident_t = cst.tile([P, P], BF)
            nc.scalar.dma_start(out=ident_t[:], in_=t_ident[:])

            # ------- resident node-feature tiles (feature-major, bf16) -----
            xT_b = res.tile([P, 2, t_c], BF)
            nc.scalar.dma_start(out=xT_b[:], in_=t_xT[:])
            h1T_b = res.tile([P, 2, t_c], BF)
            if L2_SRC == "sbuf":
                tab_sb = res.tile([P, tabcols, G], BF)

            # ================= Layer 1 =================
            for tb in range(ntg):
                # A layout: [P, gh, rel, 32 bins, 16 slots] (rel-major so the
                # dense contraction rhs per relation is contiguous)
                A_bf = pA.tile([P, 2, R, BINS_TG, SLOT], BF, tag="A")
                for bank in range(8):
                    bi = tb * 8 + bank          # bank index into streams
                    xg = pG.tile([P, 8, G], BF, tag="g")
                    nc.sync.dma_start(out=xg[:], in_=t_xs1[bi])
                    s1 = pS.tile([P, 8, P], BF, tag="S")
                    nc.sync.dma_start(out=s1[:], in_=t_s1[bi])
                    # psum cols: bin-in-bank(4) x rel(8) x slot(16)
                    aps = [psA.tile([P, 4, R, SLOT], F32, tag=f"psA{g}",
                                    name=f"apsL1_{tb}_{bank}_{g}")
                           for g in range(2)]
                    for cc in range(8):            # chunks in this bank
                        b4 = cc // 2               # bin within bank
                        for gh in range(2):
                            nc.tensor.matmul(
                                out=aps[gh][:, b4],
                                lhsT=xg[:, cc, gh * P:(gh + 1) * P],
                                rhs=s1[:, cc],
                                start=(cc == 0), stop=(cc == 7))
                    for gh in range(2):
                        nc.vector.tensor_copy(
                            out=A_bf[:, gh, :, bank * 4:(bank + 1) * 4, :],
                            in_=aps[gh][:].rearrange("p b r s -> p r b s"))

                # dense: agg1 + x@root1 + bias1  -> h1_gcn (feature-major)
                sl = slice(tb * TG, (tb + 1) * TG)
                h1g_b = pH.tile([P, 2, TG], BF, tag="h1g_b")
                aggs = [psD.tile([P, TG], F32, tag="agg",
                                 name=f"aggL1_{tb}_{hh}") for hh in range(2)]
                k = 0
                for r in range(R):
                    for gh in range(2):
                        for hh in range(2):      # interleave chains; share rhs
                            nc.tensor.matmul(
                                out=aggs[hh][:],
                                lhsT=wfull_t[:, r * 2 + gh,
                                             hh * P:(hh + 1) * P],
                                rhs=A_bf[:, gh, r],
                                start=(k == 0), stop=False)
                        k += 1
                for gh in range(2):
                    for hh in range(2):
                        nc.tensor.matmul(
                            out=aggs[hh][:],
                            lhsT=root1_t[:, gh, hh * P:(hh + 1) * P],
                            rhs=xT_b[:, gh, sl],
                            start=False, stop=(gh == 1))
                for hh in range(2):
                    nc.scalar.activation(out=h1g_b[:, hh], in_=aggs[hh][:],
                                         func=Iden, bias=bias_t[:, 0 + hh:1 + hh])
                # gate1: alpha = sigmoid([x, h1_gcn] @ g1w + g1b)
                gpss = [psD.tile([P, TG], F32, tag="agg",
                                 name=f"gpsL1_{tb}_{hh}") for hh in range(2)]
                rhs4 = [xT_b[:, 0, sl], xT_b[:, 1, sl],
                        h1g_b[:, 0], h1g_b[:, 1]]
                for k4 in range(4):
                    for hh in range(2):
                        nc.tensor.matmul(
                            out=gpss[hh][:],
                            lhsT=g1w_t[:, k4, hh * P:(hh + 1) * P],
                            rhs=rhs4[k4],
                            start=(k4 == 0), stop=(k4 == 3))
                for hh in range(2):
                    gps = gpss[hh]
                    alpha = pH.tile([P, TG], F32, tag="alpha")
                    nc.scalar.activation(out=alpha[:], in_=gps[:],
                                         func=Sigm, bias=bias_t[:, 2 + hh:3 + hh])
                    # h1 = x + alpha*(h1_gcn - x)   (bf16 residual math)
                    d = pH.tile([P, TG], BF, tag="d")
                    nc.vector.tensor_tensor(out=d[:], in0=h1g_b[:, hh],
                                            in1=xT_b[:, hh, sl], op=SUB)
                    m = pH.tile([P, TG], BF, tag="m")
                    nc.vector.tensor_tensor(out=m[:], in0=alpha[:], in1=d[:],
                                            op=MUL)
                    nc.vector.tensor_tensor(out=h1T_b[:, hh, sl], in0=m[:],
                                            in1=xT_b[:, hh, sl], op=ADD)
                # transpose h1 (bf16) to node-major rows, store blocked
                for j in range(TG // P):
                    own = pH.tile([P, G], BF, tag="own")
                    for hh in range(2):
                        tp = psT.tile([P, P], BF, tag="tp",
                                      name=f"tp_{tb}_{j}_{hh}")
                        nc.tensor.transpose(
                            out=tp[:],
                            in_=h1T_b[:, hh, tb * TG + j * P:tb * TG + (j + 1) * P],
                            identity=ident_t[:])
                        nc.scalar.copy(out=own[:, hh * P:(hh + 1) * P],
                                       in_=tp[:])
                    # Act ring: don't queue behind pending xs1/S1 stream loads
                    nc.scalar.dma_start(out=d_ownB[tb, :, j, :], in_=own[:])
                # chunked AllGather per tile-group; then pull the gathered
                # group into the SBUF table (wide 2KB/partition descriptors)
                nc.gpsimd.collective_compute(
                    "AllGather", mybir.AluOpType.bypass,
                    replica_groups=[list(range(CORES))],
                    ins=[d_ownB[tb].opt()],
                    outs=[d_tabB[tb].opt()])
                if L2_SRC == "sbuf":
                    for c in range(CORES):
                        nc.scalar.dma_start(
                            out=tab_sb[:, (tb * CORES + c) * 4:
                                       (tb * CORES + c) * 4 + 4, :],
                            in_=d_tabB[tb, c])

            # ================= Layer 2 =================
            for tb in range(ntg):
                a2ps = [psA.tile([P, 4, R, SLOT], F32, tag=f"psA{g}",
                                 name=f"apsL2_{tb}_{g}") for g in range(2)]
                for call in range(8):              # 1024-edge banks
                    ccol = tb * 512 + call * 64
                    if L2_SRC == "sbuf":
                        ng = 1024 // GSZ
                        hgTs = []
                        for gsub in range(ng):
                            hgT = pG2.tile([P, 2, GSZ], BF, tag=f"g2_{gsub}")
                            nc.gpsimd.dma_gather(
                                out_ap=hgT[:], in_ap=tab_sb[:],
                                idxs_ap=idx2_t[:, ccol + gsub * (GSZ // 16):
                                               ccol + (gsub + 1) * (GSZ // 16)],
                                num_idxs=GSZ, num_idxs_reg=GSZ, elem_size=G,
                                transpose=True, sbuf_tokens_per_rank=P,
                                sbuf_free_dim_per_rank=2 * G)
                            hgTs.append(hgT)
                    else:
                        hg = pG2.tile([P, 8, G], BF, tag="g2")
                        pre = d_tabB[0:NTG_PRE] if (call % 2 == 0
                                                     and ntg == 5) else d_tabB[:]
                        nc.gpsimd.dma_gather(
                            out_ap=hg[:],
                            in_ap=pre.rearrange("a b p k g -> (a b p k) g"),
                            idxs_ap=idx2_t[:, ccol:ccol + 64],
                            num_idxs=1024, num_idxs_reg=1024, elem_size=G)
                    bi = tb * 8 + call
                    s2 = pS.tile([P, 8, P], BF, tag="S")
                    nc.sync.dma_start(out=s2[:], in_=t_s2[bi])
                    blk = call // 2                 # 128-target block
                    for cc in range(8):
                        if L2_SRC == "sbuf":
                            # flip gathered [feat, edge] tile to edge-major
                            hsrc = hgTs[cc // (GSZ // P)]
                            hoff = (cc % (GSZ // P)) * P
                            tp2 = psT.tile([P, 2, P], BF, tag="tp2",
                                           name=f"tp2_{tb}_{call}_{cc}")
                            for fh in range(2):
                                nc.tensor.transpose(
                                    out=tp2[:, fh],
                                    in_=hsrc[:, fh, hoff:hoff + P],
                                    identity=ident_t[:])
                            em = pEM.tile([P, 2, P], BF, tag="em")
                            if cc % 2 == 0:
                                nc.vector.tensor_copy(out=em[:], in_=tp2[:])
                            else:
                                nc.scalar.copy(out=em[:], in_=tp2[:])
                            lhs_pair = (em[:, 0], em[:, 1])
                        else:
                            lhs_pair = (hg[:, cc, 0:P], hg[:, cc, P:2 * P])
                        for gh in range(2):
                            nc.tensor.matmul(
                                out=a2ps[gh][:, blk],
                                lhsT=lhs_pair[gh],
                                rhs=s2[:, cc],
                                start=(call % 2 == 0 and cc == 0),
                                stop=(call % 2 == 1 and cc == 7))
                    if call % 2 == 0:
                        continue
                    # ---- per-block epilogue: dense/gates/residual for these
                    # 128 targets run under the remaining gather calls ----
                    gsl = slice(tb * TG + blk * P, tb * TG + (blk + 1) * P)
                    A2b = pH.tile([P, 2, R, SLOT], BF, tag="A2",
                                  name=f"A2_{tb}_{blk}")
                    for gh in range(2):
                        nc.vector.tensor_copy(out=A2b[:, gh],
                                              in_=a2ps[gh][:, blk])
                    aggs2 = [psD.tile([P, TG], F32, tag="agg",
                                      name=f"aggL2_{tb}_{blk}_{hh}")
                             for hh in range(2)]
                    for gh in range(2):
                        for hh in range(2):
                            nc.tensor.matmul(
                                out=aggs2[hh][:, :P],
                                lhsT=wrel_t[:, gh, hh * P:(hh + 1) * P],
                                rhs=A2b[:, gh],
                                start=(gh == 0), stop=False)
                    for gh in range(2):
                        for hh in range(2):
                            nc.tensor.matmul(
                                out=aggs2[hh][:, :P],
                                lhsT=wroot_t[:, gh, hh * P:(hh + 1) * P],
                                rhs=h1T_b[:, gh, gsl],
                                start=False, stop=(gh == 1))
                    h2g_b = pH.tile([P, 2, P], BF, tag="h2gb",
                                    name=f"h2gb_{tb}_{blk}")
                    for hh in range(2):
                        nc.scalar.activation(out=h2g_b[:, hh],
                                             in_=aggs2[hh][:, :P], func=Iden,
                                             bias=bias_t[:, 4 + hh:5 + hh])
                    gpss2 = [psD.tile([P, TG], F32, tag="agg",
                                      name=f"gpsL2_{tb}_{blk}_{hh}")
                             for hh in range(2)]
                    rhs4b = [h1T_b[:, 0, gsl], h1T_b[:, 1, gsl],
                             h2g_b[:, 0], h2g_b[:, 1]]
                    for k4 in range(4):
                        for hh in range(2):
                            nc.tensor.matmul(
                                out=gpss2[hh][:, :P],
                                lhsT=g2w_t[:, k4, hh * P:(hh + 1) * P],
                                rhs=rhs4b[k4],
                                start=(k4 == 0), stop=(k4 == 3))
                    for hh in range(2):
                        alpha = pH.tile([P, P], F32, tag="alpha2",
                                        name=f"al2_{tb}_{blk}_{hh}")
                        nc.scalar.activation(out=alpha[:],
                                             in_=gpss2[hh][:, :P], func=Sigm,
                                             bias=bias_t[:, 6 + hh:7 + hh])
                        d = pH.tile([P, P], BF, tag="d2")
                        nc.vector.tensor_tensor(out=d[:], in0=h2g_b[:, hh],
                                                in1=h1T_b[:, hh, gsl], op=SUB)
                        m = pH.tile([P, P], F32, tag="m2")
                        nc.vector.tensor_tensor(out=m[:], in0=alpha[:],
                                                in1=d[:], op=MUL)
                        h2 = pH.tile([P, P], F32, tag="h22")
                        nc.vector.tensor_tensor(out=h2[:], in0=m[:],
                                                in1=h1T_b[:, hh, gsl], op=ADD)
                        nc.scalar.dma_start(out=t_out[hh, :, gsl],
                                            in_=h2[:])

    nc.compile()
    return nc


# ----------------------------------------------------------------------------
# host-side preprocessing + launch
# ----------------------------------------------------------------------------

def _wrap_idx(idx_pad: np.ndarray) -> np.ndarray:
    """[npad] int16 -> [128, npad/16] wrapped (i at [i%16, i//16]) + replicated."""
    w = idx_pad.reshape(-1, 16).T
    return np.ascontiguousarray(np.tile(w, (8, 1)))


def prepare(inputs: dict):
    node_features = np.asarray(inputs["node_features"], np.float32)
    edge_index = np.asarray(inputs["edge_index"], np.int64)
    edge_norm = np.asarray(inputs["edge_norm"], np.float32)
    edge_type = np.asarray(inputs["edge_type"], np.int64)
    basis = np.asarray(inputs["basis"], np.float32)
    comp = np.asarray(inputs["comp"], np.float32)
    root1 = np.asarray(inputs["root1"], np.float32)
    bias1 = np.asarray(inputs["bias1"], np.float32)
    w_rel = np.asarray(inputs["w_rel"], np.float32)
    b_rel = np.asarray(inputs["b_rel"], np.float32)
    w_root = np.asarray(inputs["w_root"], np.float32)
    gate1_w = np.asarray(inputs["gate1_w"], np.float32)
    gate1_b = np.asarray(inputs["gate1_b"], np.float32)
    gate2_w = np.asarray(inputs["gate2_w"], np.float32)
    gate2_b = np.asarray(inputs["gate2_b"], np.float32)

    src = edge_index[0].astype(np.int64)
    tgt = edge_index[1].astype(np.int64)
    rel = edge_type.astype(np.int64)

    deg = np.bincount(tgt, minlength=N)
    bins_c = -(-max(N // SLOT + 1, (E + CORES * CAP - 1) // (CORES * CAP)) // (CORES * 32)) * 32
    bins_c = max(bins_c, 32)
    packed = None
    while packed is None:
        packed = _pack_bins(deg, bins_c)
        if packed is None:
            bins_c += 32
            if bins_c > 224:
                raise RuntimeError("bin packing failed")
    bin_of, slot_of = packed
    t_c = bins_c * SLOT
    npad = bins_c * CAP
    ncol = npad // P

    core_of = bin_of // bins_c
    bin_loc = bin_of % bins_c
    tslot_of = bin_loc * SLOT + slot_of          # target slot within core
    # h1 table position: blocked group-major layout
    # sbuf gather: rank = (g*CORES + core)*4 + kk, token = tslot % 128
    # dram gather: flat row order of d_tabB is [g][c][p][kk]
    g_of = tslot_of // TG
    kk_of = (tslot_of % TG) // P
    p_of = tslot_of % P
    if L2_SRC == "sbuf":
        table_pos = ((g_of * CORES + core_of) * (TG // P) + kk_of) * P + p_of
    else:
        table_pos = ((g_of * CORES + core_of) * P + p_of) * (TG // P) + kk_of

    # per-relation mean normalization (computed from the ORIGINAL graph)
    segid = tgt * R + rel
    cnt = np.bincount(segid, minlength=N * R).astype(np.float64)
    scale_e = (1.0 / np.maximum(cnt, 1.0))[segid].astype(np.float32)

    # global edge ordering: (core, bin_loc, slot_of_tgt, rel)
    ek = np.lexsort((rel, slot_of[tgt], bin_loc[tgt], core_of[tgt]))
    e_core = core_of[tgt][ek]
    e_bin = bin_loc[tgt][ek]

    # position of each edge inside its core's padded slot array
    key = e_core.astype(np.int64) * bins_c + e_bin
    uniq, inv, counts = np.unique(key, return_inverse=True, return_counts=True)
    start = np.zeros(len(uniq), np.int64)
    np.cumsum(counts[:-1], out=start[1:])
    offs = np.arange(len(key)) - start[inv]
    if counts.max() > CAP:
        raise RuntimeError("bin overflow")
    slot_idx = e_bin * CAP + offs                 # edge slot within core

    w_full = np.einsum("rb,bio->rio", comp, basis).astype(np.float32)
    wfull_pack = np.ascontiguousarray(
        w_full.reshape(R, 2, P, G).transpose(2, 0, 1, 3).reshape(P, 16, G)
    ).astype(BF16)
    root1_pack = np.ascontiguousarray(
        root1.reshape(2, P, G).transpose(1, 0, 2)).astype(BF16)
    g1w_pack = np.ascontiguousarray(
        gate1_w.reshape(4, P, G).transpose(1, 0, 2)).astype(BF16)
    wrel_pack = np.ascontiguousarray(
        w_rel.reshape(2, P, G).transpose(1, 0, 2)).astype(BF16)
    wroot_pack = np.ascontiguousarray(
        w_root.reshape(2, P, G).transpose(1, 0, 2)).astype(BF16)
    g2w_pack = np.ascontiguousarray(
        gate2_w.reshape(4, P, G).transpose(1, 0, 2)).astype(BF16)
    bias_pack = np.stack([bias1.reshape(2, P), gate1_b.reshape(2, P),
                          b_rel.reshape(2, P), gate2_b.reshape(2, P)], 0)
    bias_pack = np.ascontiguousarray(
        bias_pack.reshape(8, P).T).astype(np.float32)   # [128, 8]
    ident = np.eye(P, dtype=np.float32).astype(BF16)
    x_bf = node_features.astype(BF16)

    in_maps = []
    for c in range(CORES):
        mask = e_core == c
        sl = slot_idx[mask]
        eidx = ek[mask]

        # per-slot arrays (npad)
        src_slot = np.zeros(npad, np.int64)        # source node per slot
        src_slot[sl] = src[eidx]
        has_edge = np.zeros(npad, bool)
        has_edge[sl] = True
        seg1 = np.zeros(npad, np.int64)
        seg1[sl] = rel[eidx] * SLOT + slot_of[tgt[eidx]]
        scl1 = np.zeros(npad, np.float32)
        scl1[sl] = scale_e[eidx]
        # L2 slot order: per 8-bin block (2048 slots = 2 gather calls),
        # edges sorted by the AG group of their source row; sources in
        # groups >2 are packed at the block tail so the first 1024-idx
        # gather call of every block only reads AG groups 0-2 (its in_ap
        # is a prefix view -> it can start before the last AG chunks).
        g_node = tslot_of // TG
        blk_e = bin_loc[tgt[eidx]] // 8
        gsrc = g_node[src[eidx]]
        late = gsrc > (NTG_PRE - 1)
        order = np.lexsort((gsrc, late, blk_e))
        e2 = eidx[order]
        blk2 = blk_e[order]
        late2 = late[order]
        nblk = npad // (2 * 1024)
        pos = np.zeros(len(e2), np.int64)
        for b in range(nblk):
            mb = blk2 == b
            n_late = int(late2[mb].sum())
            n_early = int(mb.sum()) - n_late
            assert n_late <= 1024, "late-edge overflow; raise NTG_PRE"
            pb = np.concatenate([np.arange(n_early),
                                 2048 - n_late + np.arange(n_late)])
            pos[mb] = pb
        sl2 = blk2 * 2048 + pos
        has_edge2 = np.zeros(npad, bool)
        has_edge2[sl2] = True
        seg2 = np.zeros(npad, np.int64)
        seg2[sl2] = ((bin_loc[tgt[e2]] % 8) * SLOT
                     + slot_of[tgt[e2]]).astype(np.int64)
        nrm2 = np.zeros(npad, np.float32)
        nrm2[sl2] = edge_norm[e2]
        idx2 = np.zeros(npad, np.int16)
        idx2[sl2] = table_pos[src[e2]].astype(np.int16)

        # xs1 stream: [ncol//8, 128, 8*G]; slot i -> [i//1024, i%128, (i//128%8)*G]
        xs1 = x_bf[src_slot]                       # [npad, G]
        xs1[~has_edge] = 0
        xs1 = np.ascontiguousarray(
            xs1.reshape(ncol // 8, 8, P, G).transpose(0, 2, 1, 3)
               .reshape(ncol // 8, P, 8 * G))

        # S matrices: [ncol//8, 128, 8*128]; S[slot, seg] = val
        def build_s(seg, val, msk):
            s = np.zeros((npad, P), np.float32)
            s[np.arange(npad)[msk], seg[msk]] = val[msk]
            return np.ascontiguousarray(
                s.reshape(ncol // 8, 8, P, P).transpose(0, 2, 1, 3)
                 .reshape(ncol // 8, P, 8 * P).astype(BF16))

        s1m = build_s(seg1, scl1, has_edge)
        s2m = build_s(seg2, nrm2, has_edge2)

        # x of this core's targets, feature-major [128, 2, t_c] (bf16)
        nodes_c = np.where(core_of == c)[0]
        xTc = np.zeros((t_c, G), np.float32)
        xTc[tslot_of[nodes_c]] = node_features[nodes_c]
        xT_pack = np.ascontiguousarray(
            xTc.T.reshape(2, P, t_c).transpose(1, 0, 2)).astype(BF16)

        in_maps.append({
            "xs1": xs1,
            "s1m": s1m,
            "s2m": s2m,
            "xT": xT_pack,
            "idx2": _wrap_idx(idx2),
            "wfull": wfull_pack,
            "root1": root1_pack,
            "g1w": g1w_pack,
            "wrel": wrel_pack,
            "wroot": wroot_pack,
            "g2w": g2w_pack,
            "biases": bias_pack,
            "ident": ident,
        })

    meta = (bins_c, core_of, tslot_of)
    return in_maps, meta


def postprocess(results, meta):
    bins_c, core_of, tslot_of = meta
    t_c = bins_c * SLOT
    out = np.empty((N, G), np.float32)
    for c in range(CORES):
        h2T = np.asarray(results[c]["h2T"])      # [2, 128, t_c]
        h2 = h2T.reshape(G, t_c).T               # [t_c, 256]
        nodes_c = np.where(core_of == c)[0]
        out[nodes_c] = h2[tslot_of[nodes_c]]
    return out


def run(inputs: dict, trace: bool = False):
    import time as _time
    in_maps, meta = prepare(inputs)
    bins_c = meta[0]
    if (bins_c, L2_SRC) not in _nc_cache:
        _t = _time.time()
        _nc_cache[(bins_c, L2_SRC)] = _build_nc(bins_c)
        print(f"[kernel] build+compile {_time.time() - _t:.1f}s", flush=True)
    nc = _nc_cache[(bins_c, L2_SRC)]
    _t = _time.time()
    res = run_bass_kernel_spmd(nc, in_maps, core_ids=list(range(CORES)),
                               trace=trace)
    print(f"[kernel] exec {_time.time() - _t:.1f}s", flush=True)
    out = postprocess(res.results, meta)
    return out, res


def kernel(**inputs) -> np.ndarray:
    out, _ = run(inputs, trace=False)
    return out


# revision 19
# speedup vs baseline: 1.2990x; 1.2990x over previous
"""Distributed RGCN+GraphConv (gated residual) kernel for 8 Trainium2 cores.

Sharding: target nodes are bin-packed into bins of <=16 nodes whose total
in-degree is <=256.  Each core owns BINS_C consecutive bins (graph/data
parallel over targets).  Edge lists are padded per-bin to a uniform structure
so a single SPMD NEFF serves all cores.

v3: Layer-2 source features are gathered from an SBUF-resident copy of the
AllGathered h1 table (SBUF-source transposed dma_gather) instead of per-edge
random reads from DRAM, which were HBM-latency-bound (~8.7us per 1024 rows).
The feature-major gathered tiles are flipped to edge-major with PE
transposes before the scatter-matmul aggregation.  The h1 table is stored
blocked ([group][core][partition][row][feat]) so the own-store, AllGather
and table->SBUF loads all use wide descriptors.  L1 keeps the v2 structure
(host-pregathered xs1 stream + one-hot scatter matmuls); x/h1 residuals now
run in bf16 to free SBUF for the table.
"""

import numpy as np
import ml_dtypes

import concourse.bacc as bacc
import concourse.mybir as mybir
import concourse.tile as tile
from concourse.library_config import mlp as _mlp_lib
from concourse.bass_utils import run_bass_kernel_spmd

BF16 = ml_dtypes.bfloat16

N = 20000
E = 320000
R = 8
G = 256          # feature width (g_dim == h1_dim == h2_dim)
CORES = 8
P = 128
SLOT = 16        # target slots per bin
CAP = 256        # edge slots per bin (2 chunks of 128)
TG = 512         # targets per tile-group
BINS_TG = TG // SLOT          # 32 bins per tile-group

F32 = mybir.dt.float32
BF = mybir.dt.bfloat16
I16 = mybir.dt.int16

_nc_cache: dict = {}
L2_SRC = "dram"   # "sbuf": gather h1 rows from SBUF table; "dram": from DRAM
NTG_PRE = 3       # AG groups readable by each block's first gather call
GSZ = 512         # idxs per SBUF-source gather call (transpose rx-desc limit)


# ----------------------------------------------------------------------------
# host-side: bin packing of target nodes
# ----------------------------------------------------------------------------

def _pack_bins(deg: np.ndarray, bins_c: int):
    """LPT pack nodes into CORES*bins_c bins (<=SLOT nodes, <=CAP edge sum).

    Returns (bin_of_node, slot_in_bin) or None if infeasible."""
    import heapq

    nbins = CORES * bins_c
    order = np.argsort(-deg, kind="stable")
    heap = [(0, b) for b in range(nbins)]
    heapq.heapify(heap)
    counts = np.zeros(nbins, np.int32)
    sums = np.zeros(nbins, np.int64)
    bin_of = np.full(N, -1, np.int32)
    slot_of = np.full(N, -1, np.int32)
    stash = []
    for n in order:
        d = int(deg[n])
        placed = False
        while heap:
            s, b = heapq.heappop(heap)
            if counts[b] >= SLOT:
                continue        # bin full by count; drop from heap
            if s + d > CAP:
                stash.append((s, b))
                # smallest-sum bin can't take it -> no bin can (heap is by sum)
                break
            bin_of[n] = b
            slot_of[n] = counts[b]
            counts[b] += 1
            sums[b] = s + d
            if counts[b] < SLOT:
                heapq.heappush(heap, (int(sums[b]), b))
            placed = True
            break
        for item in stash:
            heapq.heappush(heap, item)
        stash.clear()
        if not placed:
            return None
    return bin_of, slot_of


# ----------------------------------------------------------------------------
# device kernel builder (structure depends only on bins_c)
# ----------------------------------------------------------------------------

def _build_nc(bins_c: int):
    t_c = bins_c * SLOT              # targets per core
    npad = bins_c * CAP              # edge slots per core
    ncol = npad // P                 # chunk columns
    ntg = t_c // TG                  # tile groups
    nidxcol = npad // 16
    tabcols = CORES * t_c // P       # SBUF table ranks (512B each)

    nc = bacc.Bacc("TRN2", target_bir_lowering=False, debug=False,
                   num_devices=CORES)

    t_xs1 = nc.dram_tensor("xs1", [ncol // 8, P, 8 * G], BF,
                           kind="ExternalInput")
    t_s1 = nc.dram_tensor("s1m", [ncol // 8, P, 8 * P], BF,
                          kind="ExternalInput")
    t_s2 = nc.dram_tensor("s2m", [ncol // 8, P, 8 * P], BF,
                          kind="ExternalInput")
    t_xT = nc.dram_tensor("xT", [P, 2, t_c], BF, kind="ExternalInput")
    t_idx2 = nc.dram_tensor("idx2", [P, nidxcol], I16, kind="ExternalInput")
    t_wfull = nc.dram_tensor("wfull", [P, 16, G], BF, kind="ExternalInput")
    t_root1 = nc.dram_tensor("root1", [P, 2, G], BF, kind="ExternalInput")
    t_g1w = nc.dram_tensor("g1w", [P, 4, G], BF, kind="ExternalInput")
    t_wrel = nc.dram_tensor("wrel", [P, 2, G], BF, kind="ExternalInput")
    t_wroot = nc.dram_tensor("wroot", [P, 2, G], BF, kind="ExternalInput")
    t_g2w = nc.dram_tensor("g2w", [P, 4, G], BF, kind="ExternalInput")
    t_bias = nc.dram_tensor("biases", [P, 8], F32, kind="ExternalInput")
    t_ident = nc.dram_tensor("ident", [P, P], BF, kind="ExternalInput")

    t_out = nc.dram_tensor("h2T", [2, P, t_c], F32, kind="ExternalOutput")

    # blocked h1 table: own [group][128][4 rows][G], gathered
    # [group][core][128][4 rows][G] so every DRAM touch is >=2KB/partition
    d_ownB = nc.dram_tensor("h1_ownB", [ntg, P, TG // P, G], BF)
    d_tabB = nc.dram_tensor("h1_tabB", [ntg, CORES, P, TG // P, G], BF,
                            addr_space="Shared")

    Iden = mybir.ActivationFunctionType.Identity
    Sigm = mybir.ActivationFunctionType.Sigmoid
    MUL = mybir.AluOpType.mult
    SUB = mybir.AluOpType.subtract
    ADD = mybir.AluOpType.add

    with tile.TileContext(nc, num_cores=CORES) as tc:
        with tc.tile_pool(name="cst", bufs=1) as cst, \
             tc.tile_pool(name="res", bufs=1) as res, \
             tc.tile_pool(name="pA", bufs=2) as pA, \
             tc.tile_pool(name="pG", bufs=2) as pG, \
             tc.tile_pool(name="pG2", bufs=8) as pG2, \
             tc.tile_pool(name="pEM", bufs=4) as pEM, \
             tc.tile_pool(name="pS", bufs=3) as pS, \
             tc.tile_pool(name="pH", bufs=2) as pH, \
             tc.tile_pool(name="psA", bufs=2, space="PSUM") as psA, \
             tc.tile_pool(name="psD", bufs=2, space="PSUM") as psD, \
             tc.tile_pool(name="psT", bufs=2, space="PSUM") as psT:

            nc.gpsimd.load_library(_mlp_lib)

            # ------- load constants / weights (Act-engine HWDGE ring so
            # they do not head-of-line block the Sync ring's edge streams) ---
            idx2_t = cst.tile([P, nidxcol], I16)
            nc.scalar.dma_start(out=idx2_t[:], in_=t_idx2[:])
            wfull_t = cst.tile([P, 16, G], BF)
            nc.scalar.dma_start(out=wfull_t[:], in_=t_wfull[:])
            root1_t = cst.tile([P, 2, G], BF)
            nc.scalar.dma_start(out=root1_t[:], in_=t_root1[:])
            g1w_t = cst.tile([P, 4, G], BF)
            nc.scalar.dma_start(out=g1w_t[:], in_=t_g1w[:])
            wrel_t = cst.tile([P, 2, G], BF)
            nc.scalar.dma_start(out=wrel_t[:], in_=t_wrel[:])
            wroot_t = cst.tile([P, 2, G], BF)
            nc.scalar.dma_start(out=wroot_t[:], in_=t_wroot[:])
            g2w_t = cst.tile([P, 4, G], BF)
            nc.scalar.dma_start(out=g2w_t[:], in_=t_g2w[:])
            bias_t = cst.tile([P, 8], F32)
            nc.scalar.dma_start(out=bias_t[:], in_=t_bias[:])
            ident_t = cst.tile([P, P], BF)
            nc.scalar.dma_start(out=ident_t[:], in_=t_ident[:])

            # ------- resident node-feature tiles (feature-major, bf16) -----
            xT_b = res.tile([P, 2, t_c], BF)
            nc.scalar.dma_start(out=xT_b[:], in_=t_xT[:])
            h1T_b = res.tile([P, 2, t_c], BF)
            if L2_SRC == "sbuf":
                tab_sb = res.tile([P, tabcols, G], BF)

            # ================= Layer 1 =================
            for tb in range(ntg):
                # A layout: [P, gh, rel, 32 bins, 16 slots] (rel-major so the
                # dense contraction rhs per relation is contiguous)
                A_bf = pA.tile([P, 2, R, BINS_TG, SLOT], BF, tag="A")
                for bank in range(8):
                    bi = tb * 8 + bank          # bank index into streams
                    if bank % 4 == 0:           # quad-batched stream loads
                        xgq = pG.tile([P, 4, 8, G], BF, tag="g")
                        nc.sync.dma_start(
                            out=xgq[:],
                            in_=t_xs1[bi:bi + 4].rearrange("q p x -> p q x"))
                        s1q = pS.tile([P, 4, 8, P], BF, tag="S1")
                        nc.sync.dma_start(
                            out=s1q[:],
                            in_=t_s1[bi:bi + 4].rearrange("q p x -> p q x"))
                    xg = xgq[:, bank % 4]
                    s1 = s1q[:, bank % 4]
                    # psum cols: bin-in-bank(4) x rel(8) x slot(16)
                    aps = [psA.tile([P, 4, R, SLOT], F32, tag=f"psA{g}",
                                    name=f"apsL1_{tb}_{bank}_{g}")
                           for g in range(2)]
                    for cc in range(8):            # chunks in this bank
                        b4 = cc // 2               # bin within bank
                        for gh in range(2):
                            nc.tensor.matmul(
                                out=aps[gh][:, b4],
                                lhsT=xg[:, cc, gh * P:(gh + 1) * P],
                                rhs=s1[:, cc],
                                start=(cc == 0), stop=(cc == 7))
                    nc.vector.tensor_copy(
                        out=A_bf[:, 0, :, bank * 4:(bank + 1) * 4, :],
                        in_=aps[0][:].rearrange("p b r s -> p r b s"))
                    nc.scalar.copy(
                        out=A_bf[:, 1, :, bank * 4:(bank + 1) * 4, :],
                        in_=aps[1][:].rearrange("p b r s -> p r b s"))

                # dense: agg1 + x@root1 + bias1  -> h1_gcn (feature-major)
                sl = slice(tb * TG, (tb + 1) * TG)
                h1g_b = pH.tile([P, 2, TG], BF, tag="h1g_b")
                aggs = [psD.tile([P, TG], F32, tag="agg",
                                 name=f"aggL1_{tb}_{hh}") for hh in range(2)]
                k = 0
                for r in range(R):
                    for gh in range(2):
                        for hh in range(2):      # interleave chains; share rhs
                            nc.tensor.matmul(
                                out=aggs[hh][:],
                                lhsT=wfull_t[:, r * 2 + gh,
                                             hh * P:(hh + 1) * P],
                                rhs=A_bf[:, gh, r],
                                start=(k == 0), stop=False)
                        k += 1
                for gh in range(2):
                    for hh in range(2):
                        nc.tensor.matmul(
                            out=aggs[hh][:],
                            lhsT=root1_t[:, gh, hh * P:(hh + 1) * P],
                            rhs=xT_b[:, gh, sl],
                            start=False, stop=(gh == 1))
                for hh in range(2):
                    nc.scalar.activation(out=h1g_b[:, hh], in_=aggs[hh][:],
                                         func=Iden, bias=bias_t[:, 0 + hh:1 + hh])
                # gate1: alpha = sigmoid([x, h1_gcn] @ g1w + g1b)
                gpss = [psD.tile([P, TG], F32, tag="agg",
                                 name=f"gpsL1_{tb}_{hh}") for hh in range(2)]
                rhs4 = [xT_b[:, 0, sl], xT_b[:, 1, sl],
                        h1g_b[:, 0], h1g_b[:, 1]]
                for k4 in range(4):
                    for hh in range(2):
                        nc.tensor.matmul(
                            out=gpss[hh][:],
                            lhsT=g1w_t[:, k4, hh * P:(hh + 1) * P],
                            rhs=rhs4[k4],
                            start=(k4 == 0), stop=(k4 == 3))
                for hh in range(2):
                    gps = gpss[hh]
                    alpha = pH.tile([P, TG], F32, tag="alpha")
                    nc.scalar.activation(out=alpha[:], in_=gps[:],
                                         func=Sigm, bias=bias_t[:, 2 + hh:3 + hh])
                    # h1 = x + alpha*(h1_gcn - x)   (bf16 residual math)
                    d = pH.tile([P, TG], BF, tag="d")
                    nc.vector.tensor_tensor(out=d[:], in0=h1g_b[:, hh],
                                            in1=xT_b[:, hh, sl], op=SUB)
                    m = pH.tile([P, TG], BF, tag="m")
                    nc.vector.tensor_tensor(out=m[:], in0=alpha[:], in1=d[:],
                                            op=MUL)
                    nc.vector.tensor_tensor(out=h1T_b[:, hh, sl], in0=m[:],
                                            in1=xT_b[:, hh, sl], op=ADD)
                # transpose h1 (bf16) to node-major rows, store blocked
                for j in range(TG // P):
                    own = pH.tile([P, G], BF, tag="own")
                    for hh in range(2):
                        tp = psT.tile([P, P], BF, tag="tp",
                                      name=f"tp_{tb}_{j}_{hh}")
                        nc.tensor.transpose(
                            out=tp[:],
                            in_=h1T_b[:, hh, tb * TG + j * P:tb * TG + (j + 1) * P],
                            identity=ident_t[:])
                        nc.scalar.copy(out=own[:, hh * P:(hh + 1) * P],
                                       in_=tp[:])
                    # Act ring: don't queue behind pending xs1/S1 stream loads
                    nc.scalar.dma_start(out=d_ownB[tb, :, j, :], in_=own[:])
                # chunked AllGather per tile-group; then pull the gathered
                # group into the SBUF table (wide 2KB/partition descriptors)
                nc.gpsimd.collective_compute(
                    "AllGather", mybir.AluOpType.bypass,
                    replica_groups=[list(range(CORES))],
                    ins=[d_ownB[tb].opt()],
                    outs=[d_tabB[tb].opt()])
                if L2_SRC == "sbuf":
                    for c in range(CORES):
                        nc.scalar.dma_start(
                            out=tab_sb[:, (tb * CORES + c) * 4:
                                       (tb * CORES + c) * 4 + 4, :],
                            in_=d_tabB[tb, c])

            # ================= Layer 2 =================
            # Two-phase gather schedule: every 8-bin block's first 1024 edge
            # slots hold only sources from AG groups 0..NTG_PRE-1 (host
            # sorted), so all "even" gather calls use a prefix view of the
            # table and start as soon as those AG chunks land -- well before
            # the last tile-groups finish layer 1.  Their block partials are
            # parked in SBUF (bf16) and added back during the odd phase.
            a2eT = res.tile([P, ntg, 2, 4, R, SLOT], BF)
            for tb in range(ntg):
                aeps = [psA.tile([P, 4, R, SLOT], F32, tag=f"psA{g}",
                                 name=f"aepsL2_{tb}_{g}") for g in range(2)]
                for call in range(0, 8, 2):        # early 1024-edge banks
                    ccol = tb * 512 + call * 64
                    hg = pG2.tile([P, 8, G], BF, tag="g2")
                    pre = (d_h1tab[0:NTG_PRE * CORES * TG, :]
                           if ntg == 5 else d_h1tab[:])
                    nc.gpsimd.dma_gather(
                        out_ap=hg[:], in_ap=pre,
                        idxs_ap=idx2_t[:, ccol:ccol + 64],
                        num_idxs=1024, num_idxs_reg=1024, elem_size=G)
                    s2 = pS.tile([P, 8, P], BF, tag="S")
                    nc.sync.dma_start(out=s2[:], in_=t_s2[tb * 8 + call])
                    blk = call // 2                 # 128-target block
                    for cc in range(8):
                        for gh in range(2):
                            nc.tensor.matmul(
                                out=aeps[gh][:, blk],
                                lhsT=hg[:, cc, gh * P:(gh + 1) * P],
                                rhs=s2[:, cc],
                                start=(cc == 0), stop=(cc == 7))
                    for gh in range(2):
                        nc.vector.tensor_copy(out=a2eT[:, tb, gh, blk],
                                              in_=aeps[gh][:, blk])
            for tb in range(ntg):
                a2ps = [psA.tile([P, 4, R, SLOT], F32, tag=f"psA{g}",
                                 name=f"apsL2_{tb}_{g}") for g in range(2)]
                for call in range(1, 8, 2):        # late 1024-edge banks
                    ccol = tb * 512 + call * 64
                    hg = pG2.tile([P, 8, G], BF, tag="g2")
                    nc.gpsimd.dma_gather(
                        out_ap=hg[:], in_ap=d_h1tab[:],
                        idxs_ap=idx2_t[:, ccol:ccol + 64],
                        num_idxs=1024, num_idxs_reg=1024, elem_size=G)
                    s2 = pS.tile([P, 8, P], BF, tag="S")
                    nc.sync.dma_start(out=s2[:], in_=t_s2[tb * 8 + call])
                    blk = call // 2                 # 128-target block
                    for cc in range(8):
                        for gh in range(2):
                            nc.tensor.matmul(
                                out=a2ps[gh][:, blk],
                                lhsT=hg[:, cc, gh * P:(gh + 1) * P],
                                rhs=s2[:, cc],
                                start=(cc == 0), stop=(cc == 7))
                    # ---- per-block epilogue: dense/gates/residual for these
                    # 128 targets; A2 = odd-phase psum + parked even partial --
                    gsl = slice(tb * TG + blk * P, tb * TG + (blk + 1) * P)
                    A2b = pH.tile([P, 2, R, SLOT], BF, tag="A2",
                                  name=f"A2_{tb}_{blk}")
                    for gh in range(2):
                        nc.vector.tensor_tensor(out=A2b[:, gh],
                                                in0=a2ps[gh][:, blk],
                                                in1=a2eT[:, tb, gh, blk],
                                                op=ADD)
                    aggs2 = [psD.tile([P, TG], F32, tag="agg",
                                      name=f"aggL2_{tb}_{blk}_{hh}")
                             for hh in range(2)]
                    for gh in range(2):
                        for hh in range(2):
                            nc.tensor.matmul(
                                out=aggs2[hh][:, :P],
                                lhsT=wrel_t[:, gh, hh * P:(hh + 1) * P],
                                rhs=A2b[:, gh],
                                start=(gh == 0), stop=False)
                    for gh in range(2):
                        for hh in range(2):
                            nc.tensor.matmul(
                                out=aggs2[hh][:, :P],
                                lhsT=wroot_t[:, gh, hh * P:(hh + 1) * P],
                                rhs=h1T_b[:, gh, gsl],
                                start=False, stop=(gh == 1))
                    h2g_b = pH.tile([P, 2, P], BF, tag="h2gb",
                                    name=f"h2gb_{tb}_{blk}")
                    for hh in range(2):
                        nc.scalar.activation(out=h2g_b[:, hh],
                                             in_=aggs2[hh][:, :P], func=Iden,
                                             bias=bias_t[:, 4 + hh:5 + hh])
                    gpss2 = [psD.tile([P, TG], F32, tag="agg",
                                      name=f"gpsL2_{tb}_{blk}_{hh}")
                             for hh in range(2)]
                    rhs4b = [h1T_b[:, 0, gsl], h1T_b[:, 1, gsl],
                             h2g_b[:, 0], h2g_b[:, 1]]
                    for k4 in range(4):
                        for hh in range(2):
                            nc.tensor.matmul(
                                out=gpss2[hh][:, :P],
                                lhsT=g2w_t[:, k4, hh * P:(hh + 1) * P],
                                rhs=rhs4b[k4],
                                start=(k4 == 0), stop=(k4 == 3))
                    for hh in range(2):
                        alpha = pH.tile([P, P], F32, tag="alpha2",
                                        name=f"al2_{tb}_{blk}_{hh}")
                        nc.scalar.activation(out=alpha[:],
                                             in_=gpss2[hh][:, :P], func=Sigm,
                                             bias=bias_t[:, 6 + hh:7 + hh])
                        d = pH.tile([P, P], BF, tag="d2")
                        nc.vector.tensor_tensor(out=d[:], in0=h2g_b[:, hh],
                                                in1=h1T_b[:, hh, gsl], op=SUB)
                        m = pH.tile([P, P], F32, tag="m2")
                        nc.vector.tensor_tensor(out=m[:], in0=alpha[:],
                                                in1=d[:], op=MUL)
                        h2 = pH.tile([P, P], F32, tag="h22")
                        nc.vector.tensor_tensor(out=h2[:], in0=m[:],
                                                in1=h1T_b[:, hh, gsl], op=ADD)
                        nc.scalar.dma_start(out=t_out[hh, :, gsl],
                                            in_=h2[:])

    nc.compile()
    return nc


# ----------------------------------------------------------------------------
# host-side preprocessing + launch
# ----------------------------------------------------------------------------

def _wrap_idx(idx_pad: np.ndarray) -> np.ndarray:
    """[npad] int16 -> [128, npad/16] wrapped (i at [i%16, i//16]) + replicated."""
    w = idx_pad.reshape(-1, 16).T
    return np.ascontiguousarray(np.tile(w, (8, 1)))


def prepare(inputs: dict):
    node_features = np.asarray(inputs["node_features"], np.float32)
    edge_index = np.asarray(inputs["edge_index"], np.int64)
    edge_norm = np.asarray(inputs["edge_norm"], np.float32)
    edge_type = np.asarray(inputs["edge_type"], np.int64)
    basis = np.asarray(inputs["basis"], np.float32)
    comp = np.asarray(inputs["comp"], np.float32)
    root1 = np.asarray(inputs["root1"], np.float32)
    bias1 = np.asarray(inputs["bias1"], np.float32)
    w_rel = np.asarray(inputs["w_rel"], np.float32)
    b_rel = np.asarray(inputs["b_rel"], np.float32)
    w_root = np.asarray(inputs["w_root"], np.float32)
    gate1_w = np.asarray(inputs["gate1_w"], np.float32)
    gate1_b = np.asarray(inputs["gate1_b"], np.float32)
    gate2_w = np.asarray(inputs["gate2_w"], np.float32)
    gate2_b = np.asarray(inputs["gate2_b"], np.float32)

    src = edge_index[0].astype(np.int64)
    tgt = edge_index[1].astype(np.int64)
    rel = edge_type.astype(np.int64)

    deg = np.bincount(tgt, minlength=N)
    bins_c = -(-max(N // SLOT + 1, (E + CORES * CAP - 1) // (CORES * CAP)) // (CORES * 32)) * 32
    bins_c = max(bins_c, 32)
    packed = None
    while packed is None:
        packed = _pack_bins(deg, bins_c)
        if packed is None:
            bins_c += 32
            if bins_c > 224:
                raise RuntimeError("bin packing failed")
    bin_of, slot_of = packed
    t_c = bins_c * SLOT
    npad = bins_c * CAP
    ncol = npad // P

    core_of = bin_of // bins_c
    bin_loc = bin_of % bins_c
    tslot_of = bin_loc * SLOT + slot_of          # target slot within core
    # h1 table position: blocked group-major layout
    # sbuf gather: rank = (g*CORES + core)*4 + kk, token = tslot % 128
    # dram gather: flat row order of d_tabB is [g][c][p][kk]
    g_of = tslot_of // TG
    kk_of = (tslot_of % TG) // P
    p_of = tslot_of % P
    if L2_SRC == "sbuf":
        table_pos = ((g_of * CORES + core_of) * (TG // P) + kk_of) * P + p_of
    else:
        table_pos = ((g_of * CORES + core_of) * P + p_of) * (TG // P) + kk_of

    # per-relation mean normalization (computed from the ORIGINAL graph)
    segid = tgt * R + rel
    cnt = np.bincount(segid, minlength=N * R).astype(np.float64)
    scale_e = (1.0 / np.maximum(cnt, 1.0))[segid].astype(np.float32)

    # global edge ordering: (core, bin_loc, slot_of_tgt, rel)
    ek = np.lexsort((rel, slot_of[tgt], bin_loc[tgt], core_of[tgt]))
    e_core = core_of[tgt][ek]
    e_bin = bin_loc[tgt][ek]

    # position of each edge inside its core's padded slot array
    key = e_core.astype(np.int64) * bins_c + e_bin
    uniq, inv, counts = np.unique(key, return_inverse=True, return_counts=True)
    start = np.zeros(len(uniq), np.int64)
    np.cumsum(counts[:-1], out=start[1:])
    offs = np.arange(len(key)) - start[inv]
    if counts.max() > CAP:
        raise RuntimeError("bin overflow")
    slot_idx = e_bin * CAP + offs                 # edge slot within core

    w_full = np.einsum("rb,bio->rio", comp, basis).astype(np.float32)
    wfull_pack = np.ascontiguousarray(
        w_full.reshape(R, 2, P, G).transpose(2, 0, 1, 3).reshape(P, 16, G)
    ).astype(BF16)
    root1_pack = np.ascontiguousarray(
        root1.reshape(2, P, G).transpose(1, 0, 2)).astype(BF16)
    g1w_pack = np.ascontiguousarray(
        gate1_w.reshape(4, P, G).transpose(1, 0, 2)).astype(BF16)
    wrel_pack = np.ascontiguousarray(
        w_rel.reshape(2, P, G).transpose(1, 0, 2)).astype(BF16)
    wroot_pack = np.ascontiguousarray(
        w_root.reshape(2, P, G).transpose(1, 0, 2)).astype(BF16)
    g2w_pack = np.ascontiguousarray(
        gate2_w.reshape(4, P, G).transpose(1, 0, 2)).astype(BF16)
    bias_pack = np.stack([bias1.reshape(2, P), gate1_b.reshape(2, P),
                          b_rel.reshape(2, P), gate2_b.reshape(2, P)], 0)
    bias_pack = np.ascontiguousarray(
        bias_pack.reshape(8, P).T).astype(np.float32)   # [128, 8]
    ident = np.eye(P, dtype=np.float32).astype(BF16)
    x_bf = node_features.astype(BF16)

    in_maps = []
    for c in range(CORES):
        mask = e_core == c
        sl = slot_idx[mask]
        eidx = ek[mask]

        # per-slot arrays (npad)
        src_slot = np.zeros(npad, np.int64)        # source node per slot
        src_slot[sl] = src[eidx]
        has_edge = np.zeros(npad, bool)
        has_edge[sl] = True
        seg1 = np.zeros(npad, np.int64)
        seg1[sl] = rel[eidx] * SLOT + slot_of[tgt[eidx]]
        scl1 = np.zeros(npad, np.float32)
        scl1[sl] = scale_e[eidx]
        # L2 slot order: per 8-bin block (2048 slots = 2 gather calls),
        # edges sorted by the AG group of their source row; sources in
        # groups >2 are packed at the block tail so the first 1024-idx
        # gather call of every block only reads AG groups 0-2 (its in_ap
        # is a prefix view -> it can start before the last AG chunks).
        g_node = tslot_of // TG
        blk_e = bin_loc[tgt[eidx]] // 8
        gsrc = g_node[src[eidx]]
        late = gsrc > (NTG_PRE - 1)
        order = np.lexsort((gsrc, late, blk_e))
        e2 = eidx[order]
        blk2 = blk_e[order]
        late2 = late[order]
        nblk = npad // (2 * 1024)
        pos = np.zeros(len(e2), np.int64)
        for b in range(nblk):
            mb = blk2 == b
            n_late = int(late2[mb].sum())
            n_early = int(mb.sum()) - n_late
            assert n_late <= 1024, "late-edge overflow; raise NTG_PRE"
            pb = np.concatenate([np.arange(n_early),
                                 2048 - n_late + np.arange(n_late)])
            pos[mb] = pb
        sl2 = blk2 * 2048 + pos
        has_edge2 = np.zeros(npad, bool)
        has_edge2[sl2] = True
        seg2 = np.zeros(npad, np.int64)
        seg2[sl2] = ((bin_loc[tgt[e2]] % 8) * SLOT
                     + slot_of[tgt[e2]]).astype(np.int64)
        nrm2 = np.zeros(npad, np.float32)
        nrm2[sl2] = edge_norm[e2]
        idx2 = np.zeros(npad, np.int16)
        idx2[sl2] = table_pos[src[e2]].astype(np.int16)

        # xs1 stream: [ncol//8, 128, 8*G]; slot i -> [i//1024, i%128, (i//128%8)*G]
        xs1 = x_bf[src_slot]                       # [npad, G]
        xs1[~has_edge] = 0
        xs1 = np.ascontiguousarray(
            xs1.reshape(ncol // 8, 8, P, G).transpose(0, 2, 1, 3)
               .reshape(ncol // 8, P, 8 * G))

        # S matrices: [ncol//8, 128, 8*128]; S[slot, seg] = val
        def build_s(seg, val, msk):
            s = np.zeros((npad, P), np.float32)
            s[np.arange(npad)[msk], seg[msk]] = val[msk]
            return np.ascontiguousarray(
                s.reshape(ncol // 8, 8, P, P).transpose(0, 2, 1, 3)
                 .reshape(ncol // 8, P, 8 * P).astype(BF16))

        s1m = build_s(seg1, scl1, has_edge)
        s2m = build_s(seg2, nrm2, has_edge2)

        # x of this core's targets, feature-major [128, 2, t_c] (bf16)
        nodes_c = np.where(core_of == c)[0]
        xTc = np.zeros((t_c, G), np.float32)
        xTc[tslot_of[nodes_c]] = node_features[nodes_c]
        xT_pack = np.ascontiguousarray(
            xTc.T.reshape(2, P, t_c).transpose(1, 0, 2)).astype(BF16)

        in_maps.append({
            "xs1": xs1,
            "s1m": s1m,
            "s2m": s2m,
            "xT": xT_pack,
            "idx2": _wrap_idx(idx2),
            "wfull": wfull_pack,
            "root1": root1_pack,
            "g1w": g1w_pack,
            "wrel": wrel_pack,
            "wroot": wroot_pack,
            "g2w": g2w_pack,
            "biases": bias_pack,
            "ident": ident,
        })

    meta = (bins_c, core_of, tslot_of)
    return in_maps, meta


def postprocess(results, meta):
    bins_c, core_of, tslot_of = meta
    t_c = bins_c * SLOT
    out = np.empty((N, G), np.float32)
    for c in range(CORES):
        h2T = np.asarray(results[c]["h2T"])      # [2, 128, t_c]
        h2 = h2T.reshape(G, t_c).T               # [t_c, 256]
        nodes_c = np.where(core_of == c)[0]
        out[nodes_c] = h2[tslot_of[nodes_c]]
    return out


def run(inputs: dict, trace: bool = False):
    import time as _time
    in_maps, meta = prepare(inputs)
    bins_c = meta[0]
    if (bins_c, L2_SRC) not in _nc_cache:
        _t = _time.time()
        _nc_cache[(bins_c, L2_SRC)] = _build_nc(bins_c)
        print(f"[kernel] build+compile {_time.time() - _t:.1f}s", flush=True)
    nc = _nc_cache[(bins_c, L2_SRC)]
    _t = _time.time()
    res = run_bass_kernel_spmd(nc, in_maps, core_ids=list(range(CORES)),
                               trace=trace)
    print(f"[kernel] exec {_time.time() - _t:.1f}s", flush=True)
    out = postprocess(res.results, meta)
    return out, res


def kernel(**inputs) -> np.ndarray:
    out, _ = run(inputs, trace=False)
    return out
